# revision 29
# baseline (speedup 1.0000x reference)
"""Trainium2 Bass kernel for nn_Attention_45724221833663 (sparse_attention).

Strategy: data-parallel over batch B=8 across the 8 NeuronCores (one batch
element per core). All matmuls run in bf16 with fp32 PSUM accumulation.

Per-core dataflow (all layouts chosen to avoid on-chip transposes of large
activations; weights and x are transposed on the host while sharding, and
ln_g/ln_b are folded into Wp/bp on the host):
  xcatT  [c=1024, kvp=1152]  (= concat(x_text, x).T, zero-padded 1101->1152)
  vw     [kvp, h, 65] = (xcatT.T @ WvT) interleaved per head + ones column
  qT     [o, n]    = WqT.T @ xT          (o = head-major channel)
  kT     [o, kvp]  = WkT.T @ xcatT
  per head pair (even head on PE row-tile 0, odd head on row-tile 64):
    scores for both heads of the pair go into ONE [128,1024] psum tile per
    (kv-tile, n-half): cols 0:512 = even head (psum bank A), cols 512:1024 =
    odd head (bank B).  The two matmuls use disjoint PE row groups and
    disjoint psum banks and share one psum-reuse dependency, so the
    hardware runs them concurrently (2x PE throughput at the K=64
    contraction).
    E = exp(scoresT / 8)     (ScalarE, one [rows,1024] activation per psum
                             tile, n-half-major); row kv=0 and the pad rows
                             are zeroed
    avp[n,0:65] = sum_kv E[kv,n-tile] * vw[kv, h, :]   (col 64 = S[n])
    attn[n, h*64:+64] = avp[:, :64] * (1/S) + tanh(g_h) * v_h[kv=0]
  LayerNorm over channels (rows of attn, bf16 input like the reference's
  bf16 cast; ln_g/ln_b pre-folded).  The final pair's AV runs n-tile-major
  with the LN chain and the output projection interleaved at depth 2.  In
  the no-bias fast path L = (x - mu) only and rstd is applied as the
  per-partition scale of the psum-evacuation activation, so the
  projection never waits on the sqrt (whose activation-table load can
  only happen after the last exp).  With a nonzero folded bias, LN is
  computed in full and the bias is added as a rank-1 matmul.
"""

import os
import numpy as np
import ml_dtypes

import concourse.bacc as bacc
import concourse.tile as tile
from concourse import mybir
from concourse.masks import make_identity
from concourse.bass_utils import run_bass_kernel_spmd

F32 = mybir.dt.float32
BF16 = mybir.dt.bfloat16
AF = mybir.ActivationFunctionType
OP = mybir.AluOpType

B, N, P, DIM, H = 8, 1024, 77, 1024, 16
HD = DIM // H          # 64
KV = P + N             # 1101
KT = 9                 # kv tiles of 128
KVP = KT * 128         # 1152 padded
NT = N // 128          # 8 n tiles
CC = DIM // 128        # 8 contraction chunks
OT = DIM // 128        # 8 output-channel tiles
LN_EPS = 1e-5

LAST_EXEC_NS = None
_CACHE = {}


def _emit(tc, with_bias):
    nc = tc.nc

    xcatT_d = nc.dram_tensor("xcatT", [DIM, KVP], BF16, kind="ExternalInput").ap()
    wq_d = nc.dram_tensor("wqT", [DIM, DIM], BF16, kind="ExternalInput").ap()
    wk_d = nc.dram_tensor("wkT", [DIM, DIM], BF16, kind="ExternalInput").ap()
    wv_d = nc.dram_tensor("wvT", [DIM, DIM], BF16, kind="ExternalInput").ap()
    wp_d = nc.dram_tensor("wpT", [DIM, DIM], BF16, kind="ExternalInput").ap()
    tanhg_d = nc.dram_tensor("tanhg", [1, H], F32, kind="ExternalInput").ap()
    bp_d = nc.dram_tensor("bp_bf", [1, DIM], BF16, kind="ExternalInput").ap()
    out_d = nc.dram_tensor("out", [N, DIM], F32, kind="ExternalOutput").ap()

    xcat_re = xcatT_d.rearrange("(j p) f -> p j f", p=128)
    wq_re = wq_d.rearrange("(j p) o -> p j o", p=128)
    wk_re = wk_d.rearrange("(j p) o -> p j o", p=128)
    wv_re = wv_d.rearrange("(j p) o -> p j o", p=128)
    wp_re = wp_d.rearrange("(j p) o -> p j o", p=128)

    from contextlib import ExitStack

    with ExitStack() as top:
        consts = top.enter_context(tc.tile_pool(name="consts", bufs=1))
        acts = top.enter_context(tc.tile_pool(name="acts", bufs=1))
        ph1 = top.enter_context(tc.tile_pool(name="ph1", bufs=1))
        wstream = top.enter_context(tc.tile_pool(name="wstream", bufs=3))
        qkp = top.enter_context(tc.tile_pool(name="qkp", bufs=3))
        epool = top.enter_context(tc.tile_pool(name="epool", bufs=3))
        tpool = top.enter_context(tc.tile_pool(name="tmp", bufs=4))
        ltp = top.enter_context(tc.tile_pool(name="ltp", bufs=6))
        opool = top.enter_context(tc.tile_pool(name="outp", bufs=3))
        ps_proj = top.enter_context(tc.tile_pool(name="ps_proj", bufs=2, space="PSUM"))
        ps_scores = top.enter_context(
            tc.tile_pool(name="ps_scores", bufs=2, space="PSUM"))
        ps_av = top.enter_context(tc.tile_pool(name="ps_av", bufs=2, space="PSUM"))

        # ---- constants ----
        tanhg_sb = consts.tile([128, H], F32, tag="tanhg")
        if with_bias:
            bp_sb = consts.tile([1, DIM], BF16, tag="bp")
            nc.sync.dma_start(out=bp_sb, in_=bp_d)
            ones1 = consts.tile([1, 128], BF16, tag="ones1")
            nc.gpsimd.memset(ones1, 1.0)
        eps_t = consts.tile([128, 1], F32, tag="eps")
        nc.vector.memset(eps_t, LN_EPS)
        ident = consts.tile([128, 128], BF16, tag="ident")
        make_identity(nc, ident)

        # ---- persistent activations ----
        vw_sb = acts.tile([128, KT, H, HD + 1], BF16, tag="vw")  # [kv-part, kv-tile, h, d+1]
        attn_sb = acts.tile([128, NT, H, HD], BF16, tag="attn")  # [n-part, n-tile, h, d]

        # input loads, c-chunk granular; only xcatT is loaded up front --
        # wv/wp loads are emitted later, in consumption order
        xcatT_sb = ph1.tile([128, CC, KVP], BF16, tag="xcatT")
        # wv and wp share one slot: wv dies after the v projection, wp is
        # only needed from the output projection onwards
        wv_sb = ph1.tile([128, CC, DIM], BF16, tag="wvwp")
        # descriptor issue on an engine queue costs ~600ns each; spread the
        # startup-critical loads over the three DMA-capable queues, early
        # xcat chunks on the queues that clear their start barrier first
        w0q = wstream.tile([128, CC, 128], BF16, tag="w")
        nc.scalar.dma_start(out=w0q, in_=wq_re[:, :, 0:128])
        w0k = wstream.tile([128, CC, 128], BF16, tag="w")
        nc.scalar.dma_start(out=w0k, in_=wk_re[:, :, 0:128])
        xcat_q = [nc.gpsimd, nc.gpsimd, nc.gpsimd, nc.gpsimd,
                  nc.scalar, nc.sync, nc.sync, nc.sync]
        for cc in range(CC):
            xcat_q[cc].dma_start(out=xcatT_sb[:, cc, :], in_=xcat_re[:, cc, :])

        # ---- q/k projections interleaved with their dependent head pairs,
        # so ScalarE (exp) fills while PE still runs projections ----
        last_rows = KV - (KT - 1) * 128  # 77
        ksplits = [(0, 512), (512, 512), (1024, KV - 1024)]

        def emit_vproj(kvts):
            # v projection into vw (head-interleaved), natural [kv, o] layout
            for kvt in kvts:
                for half in range(2):
                    ps = ps_proj.tile([128, 512], F32, tag="ps")
                    for cc in range(CC):
                        nc.tensor.matmul(
                            ps,
                            xcatT_sb[:, cc, kvt * 128:(kvt + 1) * 128],
                            wv_sb[:, cc, half * 512:(half + 1) * 512],
                            start=(cc == 0),
                            stop=(cc == CC - 1),
                        )
                    nc.vector.tensor_copy(
                        vw_sb[:, kvt, half * 8:(half + 1) * 8, 0:HD],
                        ps.rearrange("p (h d) -> p h d", d=HD),
                    )

        def emit_qk(ot, wtq=None, wtk=None):
            qt = qkp.tile([128, N], BF16, tag="qt")
            kt = qkp.tile([128, KVP], BF16, tag="kt")
            # pad keys (kv 1101:1152) are zero; scores psum partitions for
            # them are never read by the exp, but zero them for the checker
            nc.gpsimd.memset(kt[:, KV:KVP], 0.0)
            if wtq is None:
                wtq = wstream.tile([128, CC, 128], BF16, tag="w")
                nc.sync.dma_start(out=wtq, in_=wq_re[:, :, ot * 128:(ot + 1) * 128])
            for half in range(2):
                ps = ps_proj.tile([128, 512], F32, tag="ps")
                for cc in range(CC):
                    nc.tensor.matmul(
                        ps,
                        wtq[:, cc, :],
                        xcatT_sb[:, cc, P + half * 512: P + (half + 1) * 512],
                        start=(cc == 0),
                        stop=(cc == CC - 1),
                    )
                nc.vector.tensor_copy(qt[:, half * 512:(half + 1) * 512], ps)
            if wtk is None:
                wtk = wstream.tile([128, CC, 128], BF16, tag="w")
                nc.sync.dma_start(out=wtk, in_=wk_re[:, :, ot * 128:(ot + 1) * 128])
            for off, width in ksplits:
                ps = ps_proj.tile([128, 512], F32, tag="ps")
                for cc in range(CC):
                    nc.tensor.matmul(
                        ps[:, :width],
                        wtk[:, cc, :],
                        xcatT_sb[:, cc, off:off + width],
                        start=(cc == 0),
                        stop=(cc == CC - 1),
                    )
                nc.vector.tensor_copy(kt[:, off:off + width], ps[:, :width])
            return qt, kt

        def emit_scores_pair(qt, kt):
            # Scores for the even/odd head pair.  Both heads of a (kv-tile,
            # n-half) share ONE [128,1024] psum tile: even head -> cols
            # 0:512 (bank A) on PE row-tile 0, odd head -> cols 512:1024
            # (bank B) on row-tile 64.  Disjoint row groups + disjoint psum
            # banks + a shared psum-reuse dependency means the two matmuls
            # issue back-to-back and execute concurrently on the PE.
            # E layout: [kv-part, kv-tile, n-half, head, 512].
            e2 = epool.tile([128, KT, 2, 2, 512], BF16, tag="e")
            nc.gpsimd.memset(e2[:, KT - 1], 0.0)
            # half-major order so the AV of n-tiles 0-3 (which only needs
            # half 0) can start as soon as half 0's exps have drained
            for half in range(2):
                for kvt in range(KT):
                    rows = last_rows if kvt == KT - 1 else 128
                    ps = ps_scores.tile([128, 1024], F32, tag="pss")
                    nc.tensor.matmul(
                        ps[:, 0:512],
                        kt[0:64, kvt * 128:(kvt + 1) * 128],
                        qt[0:64, half * 512:(half + 1) * 512],
                        start=True, stop=True,
                    )
                    nc.tensor.matmul(
                        ps[:, 512:1024],
                        kt[64:128, kvt * 128:(kvt + 1) * 128],
                        qt[64:128, half * 512:(half + 1) * 512],
                        start=True, stop=True,
                    )
                    nc.scalar.activation(
                        e2[:rows, kvt, half], ps[:rows], AF.Exp,
                        bias=0.0, scale=0.125)
            # first key column is gated separately; one memset per n-half so
            # the AV of n-tiles 0-3 only depends on half 0's exps
            nc.gpsimd.memset(e2[0:1, 0, 0], 0.0)
            nc.gpsimd.memset(e2[0:1, 0, 1], 0.0)
            return e2

        def eslice(e2, hh, kvt, nt):
            q, r = divmod(nt, 4)
            return e2[:, kvt, q, hh, r * 128:(r + 1) * 128]

        def emit_gv0(h):
            gv0 = tpool.tile([128, HD], BF16, tag="gv0")
            nc.gpsimd.partition_broadcast(gv0, vw_sb[0:1, 0, h, 0:HD])
            gv0s = tpool.tile([128, HD], F32, tag="gv0s")
            nc.vector.tensor_scalar_mul(gv0s, gv0, tanhg_sb[:, h:h + 1])
            return gv0s

        def emit_av_nt(h, e2, hh, nt, gv0s, alt, fixup_engine=None):
            if alt and nt % 2 == 1:
                avp = ps_proj.tile([128, HD + 1], F32, tag="ps")
            else:
                avp = ps_av.tile([128, HD + 1], F32, tag="avp")
            for kvt in range(KT):
                nc.tensor.matmul(
                    avp,
                    eslice(e2, hh, kvt, nt),
                    vw_sb[:, kvt, h, :],
                    start=(kvt == 0),
                    stop=(kvt == KT - 1),
                )
            rs = tpool.tile([128, 1], F32, tag="rs")
            nc.vector.reciprocal(rs, avp[:, HD:HD + 1])
            (fixup_engine or nc.vector).scalar_tensor_tensor(
                out=attn_sb[:, nt, h, :],
                in0=avp[:, 0:HD],
                scalar=rs,
                in1=gv0s,
                op0=OP.mult,
                op1=OP.add,
            )

        def emit_head_tail(h, e2, hh, alt=False):
            """Everything after E for head h: gate prep, AV + fixup.
            alt=True additionally cycles the (by-then idle) proj psum pool
            for deeper AV pipelining on the final head pairs."""
            gv0s = emit_gv0(h)
            for nt in range(NT):
                emit_av_nt(h, e2, hh, nt, gv0s, alt)

        # ---- LayerNorm per n-tile (consumed later by the projection) ----
        def emit_ln(nt):
            xa = attn_sb[:, nt].rearrange("p h d -> p (h d)")
            xs = xa.rearrange("p (s f) -> p s f", f=512)
            stats = tpool.tile([128, 2, 6], F32, tag="stats")
            for s in range(2):
                nc.vector.bn_stats(stats[:, s, :], xs[:, s, :])
            mv = tpool.tile([128, 2], F32, tag="mv")
            nc.vector.bn_aggr(mv, stats)
            rstd = tpool.tile([128, 1], F32, tag="rstd")
            nc.scalar.activation(rstd, mv[:, 1:2], AF.Sqrt, bias=eps_t, scale=1.0)
            nc.vector.reciprocal(rstd, rstd)
            L_t = qkp.tile([128, DIM], BF16, tag="qt")
            if with_bias:
                # ln_g/ln_b are folded into Wp/bp: L = (x - mu) * rstd
                nc.vector.tensor_scalar(
                    out=L_t, in0=xa, scalar1=mv[:, 0:1], scalar2=rstd,
                    op0=OP.subtract, op1=OP.mult,
                )
                return L_t, None
            # L = x - mu only; rstd is applied as the psum-evacuation scale
            # (scalar per n row distributes over the contraction), so the
            # transposes/matmuls don't wait on the sqrt -- whose activation
            # table load can only happen after the last exp
            nc.vector.tensor_scalar_sub(L_t, xa, mv[:, 0:1])
            return L_t, rstd

        # pair 0's scores/exp are hoisted before the v projection so ScalarE
        # starts as early as possible
        # software pipeline: scores/exp run one head-pair ahead of the
        # AV/fixup tails so ScalarE never starves
        pend = []
        qt0, kt0 = emit_qk(0, w0q, w0k)
        # split sync/scalar: one queue alone delivers the last chunk too
        # late for the v-projection (keep gpsimd free for the e2 memsets;
        # the scalar queue's exp stream only starts at the first pair)
        for cc in range(CC):
            dmae = nc.sync if cc % 2 == 0 else nc.scalar
            dmae.dma_start(out=wv_sb[:, cc, :], in_=wv_re[:, cc, :])
        pend.append(emit_scores_pair(qt0, kt0))
        qt, kt = emit_qk(1)
        pend.append(emit_scores_pair(qt, kt))
        # ones column for the row-sum S (E rows for kv=0/pad are zeroed);
        # disjoint from the v-projection's columns, so set it up front
        nc.gpsimd.memset(vw_sb[:, :, :, HD:HD + 1], 1.0)
        # tanh(gate) broadcast: first consumer is the head tails ~90us in,
        # so keep it off the startup-critical queues (and off scalar, whose
        # queue is the exp stream by this point)
        nc.sync.dma_start(out=tanhg_sb, in_=tanhg_d.to_broadcast([128, H]))
        emit_vproj(range(KT))
        wp_sb = ph1.tile([128, CC, DIM], BF16, tag="wvwp")
        for cc in range(CC):
            nc.sync.dma_start(out=wp_sb[:, cc, :], in_=wp_re[:, cc, :])
        done = 0
        for ot in range(2, OT):
            qt, kt = emit_qk(ot)
            pend.append(emit_scores_pair(qt, kt))
            ep = pend.pop(0)
            emit_head_tail(2 * done, ep, 0, alt=(ot >= OT - 2))
            emit_head_tail(2 * done + 1, ep, 1, alt=(ot >= OT - 2))
            done += 1
        def emit_outproj(nt, L_t, rstd):
            # transpose LN rows then project: out[n, o] = L @ Wp'.T + bp'
            # pp accumulators alternate ps_proj/ps_scores from nt 2 on (the
            # scores pool is dead once the exp stream drains) -- a 4-slot
            # rotation so each projection never waits on the previous
            # n-tile's psum evacuation
            if nt >= 2 and nt % 2 == 1:
                pp0 = ps_scores.tile([128, 512], F32, tag="pss")
                pp1 = ps_scores.tile([128, 512], F32, tag="pss")
            else:
                pp0 = ps_proj.tile([128, 512], F32, tag="ps")
                pp1 = ps_proj.tile([128, 512], F32, tag="ps")
            for cc in range(CC):
                # ps_av only: ps_scores' slots still have WAR dependencies
                # on the final pair's exp, which would stall early n-tiles
                pst = ps_av.tile([128, 128], BF16, tag="avp")
                nc.tensor.transpose(
                    pst, L_t[:, cc * 128:(cc + 1) * 128], ident
                )
                ltc = ltp.tile([128, 128], BF16, tag="ltc")
                # DVE only: ScalarE's in-order queue is saturated with exps
                # while the first projections run -- a copy waiting on its
                # transpose there would stall the whole exp stream
                nc.vector.tensor_copy(ltc, pst)
                nc.tensor.matmul(
                    pp0, ltc, wp_sb[:, cc, 0:512],
                    start=(cc == 0), stop=(not with_bias and cc == CC - 1),
                )
                nc.tensor.matmul(
                    pp1, ltc, wp_sb[:, cc, 512:1024],
                    start=(cc == 0), stop=(not with_bias and cc == CC - 1),
                )
            if with_bias:
                # bias as rank-1 accumulation (PSUM is not a legal DMA
                # source, so stage through SBUF)
                nc.tensor.matmul(pp0, ones1, bp_sb[:, 0:512], start=False, stop=True)
                nc.tensor.matmul(pp1, ones1, bp_sb[:, 512:1024], start=False, stop=True)
            ot0 = opool.tile([128, 512], F32, tag="ot")
            ot1 = opool.tile([128, 512], F32, tag="ot")
            if rstd is None:
                nc.scalar.copy(out=ot0, in_=pp0)
                nc.scalar.copy(out=ot1, in_=pp1)
            else:
                nc.scalar.activation(ot0, pp0, AF.Copy, bias=0.0, scale=rstd)
                nc.scalar.activation(ot1, pp1, AF.Copy, bias=0.0, scale=rstd)
            nc.sync.dma_start(out=out_d[nt * 128:(nt + 1) * 128, 0:512], in_=ot0)
            nc.sync.dma_start(out=out_d[nt * 128:(nt + 1) * 128, 512:1024], in_=ot1)

        ep = pend.pop(0)
        emit_head_tail(2 * done, ep, 0, alt=True)
        emit_head_tail(2 * done + 1, ep, 1, alt=True)
        done += 1
        # final pair: nt-major AV with the LayerNorm chain and the output
        # projection of the previous n-tile interleaved, so the PE stays on
        # projection matmuls while the DVE runs the LN chain of n-tile nt.
        # AV psum comes only from ps_av here; ps_proj is cycling pp0/pp1.
        ep = pend.pop(0)
        gv0s_e = emit_gv0(2 * done)
        gv0s_o = emit_gv0(2 * done + 1)
        # depth-2 pipeline: the LN chain of n-tile nt has the AV of nt+1
        # plus the projection of nt-1 of PE time to complete before the
        # projection of nt needs it -- depth 1 stalls the PE on the DVE
        # every n-tile and drops it out of max p-state
        L_q = []
        for nt in range(NT):
            emit_av_nt(2 * done, ep, 0, nt, gv0s_e, alt=False)
            emit_av_nt(2 * done + 1, ep, 1, nt, gv0s_o, alt=False)
            L_t, rstd = emit_ln(nt)
            L_q.append((nt, L_t, rstd))
            if len(L_q) > 2:
                emit_outproj(*L_q.pop(0))
        for item in L_q:
            emit_outproj(*item)


def build_program(with_bias=False):
    key = ("nc", with_bias)
    if key in _CACHE:
        return _CACHE[key]
    nc = bacc.Bacc("TRN2", target_bir_lowering=False, debug=False, num_devices=8,
                   enable_partition_id=False)
    with tile.TileContext(nc) as tc:
        _emit(tc, with_bias)
    nc.compile()
    _CACHE[key] = nc
    return nc


def prep_inputs(x, x_text, Wq, Wk, Wv, gate, ln_g, ln_b, Wp, bp):
    """Host-side sharding/layout prep. Returns the 8 per-core input maps."""
    bf = ml_dtypes.bfloat16
    x = np.asarray(x, np.float32)
    x_text = np.asarray(x_text, np.float32)
    xcat = np.concatenate([x_text, x], axis=1)          # [B, KV, DIM]
    xcatT = np.zeros((B, DIM, KVP), np.float32)
    xcatT[:, :, :KV] = xcat.transpose(0, 2, 1)
    xcatT = xcatT.astype(bf)
    wqT = np.ascontiguousarray(np.asarray(Wq, np.float32).T).astype(bf)
    wkT = np.ascontiguousarray(np.asarray(Wk, np.float32).T).astype(bf)
    wvT = np.ascontiguousarray(np.asarray(Wv, np.float32).T).astype(bf)
    # fold LayerNorm affine into the output projection:
    #   (L*g + b) @ Wp.T + bp == L @ (Wp*g).T + (bp + Wp @ b)
    Wp = np.asarray(Wp, np.float32)
    g = np.asarray(ln_g, np.float32).reshape(DIM)
    bvec = np.asarray(ln_b, np.float32).reshape(DIM)
    Wpf = Wp * g[None, :]
    bpf = np.asarray(bp, np.float32).reshape(DIM) + Wp @ bvec
    wpT = np.ascontiguousarray(Wpf.T).astype(bf)
    tanhg = np.tanh(np.asarray(gate, np.float32)).reshape(1, H).astype(np.float32)
    bp_bf = bpf.reshape(1, DIM).astype(bf)
    in_maps = []
    for b in range(B):
        in_maps.append({
            "xcatT": np.ascontiguousarray(xcatT[b]),
            "wqT": wqT, "wkT": wkT, "wvT": wvT, "wpT": wpT,
            "tanhg": tanhg, "bp_bf": bp_bf,
        })
    return in_maps


def kernel(**inputs):
    global LAST_EXEC_NS
    in_maps = prep_inputs(**inputs)
    with_bias = bool(np.any(in_maps[0]["bp_bf"]))
    nc = build_program(with_bias)
    trace = bool(int(os.environ.get("BASS_TRACE_RUN", "0")))
    res = run_bass_kernel_spmd(
        nc, in_maps, core_ids=list(range(8)), trace=trace,
    )
    LAST_EXEC_NS = res.exec_time_ns
    out = np.stack([r["out"] for r in res.results], axis=0)
    return out.astype(np.float32)


# revision 30
# speedup vs baseline: 1.5242x; 1.5242x over previous
"""Trainium2 Bass kernel for nn_Attention_45724221833663 (sparse_attention).

Strategy: data-parallel over batch B=8 across the 8 NeuronCores (one batch
element per core). All matmuls run in bf16 with fp32 PSUM accumulation.

Per-core dataflow (all layouts chosen to avoid on-chip transposes of large
activations; weights and x are transposed on the host while sharding, and
ln_g/ln_b are folded into Wp/bp on the host):
  xcatT  [c=1024, kvp=1152]  (= concat(x_text, x).T, zero-padded 1101->1152)
  vw     [kvp, h, 65] = (xcatT.T @ WvT) interleaved per head + ones column
  qT     [o, n]    = WqT.T @ xT          (o = head-major channel)
  kT     [o, kvp]  = WkT.T @ xcatT
  per head pair (even head on PE row-tile 0, odd head on row-tile 64):
    scores for both heads of the pair go into ONE [128,1024] psum tile per
    (kv-tile, n-half): cols 0:512 = even head (psum bank A), cols 512:1024 =
    odd head (bank B).  The two matmuls use disjoint PE row groups and
    disjoint psum banks and share one psum-reuse dependency, so the
    hardware runs them concurrently (2x PE throughput at the K=64
    contraction).
    E = exp(scoresT / 8)     (ScalarE, one [rows,1024] activation per psum
                             tile, n-half-major); row kv=0 and the pad rows
                             are zeroed
    avp[n,0:65] = sum_kv E[kv,n-tile] * vw[kv, h, :]   (col 64 = S[n])
    attn[n, h*64:+64] = avp[:, :64] * (1/S) + tanh(g_h) * v_h[kv=0]
  LayerNorm over channels (rows of attn, bf16 input like the reference's
  bf16 cast; ln_g/ln_b pre-folded).  The final pair's AV runs n-tile-major
  with the LN chain and the output projection interleaved at depth 2.  In
  the no-bias fast path L = (x - mu) only and rstd is applied as the
  per-partition scale of the psum-evacuation activation, so the
  projection never waits on the sqrt (whose activation-table load can
  only happen after the last exp).  With a nonzero folded bias, LN is
  computed in full and the bias is added as a rank-1 matmul.
"""

import os
import numpy as np
import ml_dtypes

import concourse.bacc as bacc
import concourse.tile as tile
from concourse import mybir
from concourse.masks import make_identity
from concourse.bass_utils import run_bass_kernel_spmd

F32 = mybir.dt.float32
BF16 = mybir.dt.bfloat16
AF = mybir.ActivationFunctionType
OP = mybir.AluOpType

B, N, P, DIM, H = 8, 1024, 77, 1024, 16
HD = DIM // H          # 64
KV = P + N             # 1101
KT = 9                 # kv tiles of 128
KVP = KT * 128         # 1152 padded
NT = N // 128          # 8 n tiles
CC = DIM // 128        # 8 contraction chunks
OT = DIM // 128        # 8 output-channel tiles
LN_EPS = 1e-5

LAST_EXEC_NS = None
_CACHE = {}


def _emit(tc, with_bias):
    nc = tc.nc

    xcatT_d = nc.dram_tensor("xcatT", [DIM, KVP], BF16, kind="ExternalInput").ap()
    wq_d = nc.dram_tensor("wqT", [DIM, DIM], BF16, kind="ExternalInput").ap()
    wk_d = nc.dram_tensor("wkT", [DIM, DIM], BF16, kind="ExternalInput").ap()
    wv_d = nc.dram_tensor("wvT", [DIM, DIM], BF16, kind="ExternalInput").ap()
    wp_d = nc.dram_tensor("wpT", [DIM, DIM], BF16, kind="ExternalInput").ap()
    tanhg_d = nc.dram_tensor("tanhg", [1, H], F32, kind="ExternalInput").ap()
    bp_d = nc.dram_tensor("bp_bf", [1, DIM], BF16, kind="ExternalInput").ap()
    out_d = nc.dram_tensor("out", [N, DIM], F32, kind="ExternalOutput").ap()

    xcat_re = xcatT_d.rearrange("(j p) f -> p j f", p=128)
    wq_re = wq_d.rearrange("(j p) o -> p j o", p=128)
    wk_re = wk_d.rearrange("(j p) o -> p j o", p=128)
    wv_re = wv_d.rearrange("(j p) o -> p j o", p=128)
    wp_re = wp_d.rearrange("(j p) o -> p j o", p=128)

    from contextlib import ExitStack

    with ExitStack() as top:
        consts = top.enter_context(tc.tile_pool(name="consts", bufs=1))
        acts = top.enter_context(tc.tile_pool(name="acts", bufs=1))
        ph1 = top.enter_context(tc.tile_pool(name="ph1", bufs=1))
        wstream = top.enter_context(tc.tile_pool(name="wstream", bufs=3))
        qkp = top.enter_context(tc.tile_pool(name="qkp", bufs=3))
        epool = top.enter_context(tc.tile_pool(name="epool", bufs=3))
        tpool = top.enter_context(tc.tile_pool(name="tmp", bufs=4))
        ltp = top.enter_context(tc.tile_pool(name="ltp", bufs=6))
        opool = top.enter_context(tc.tile_pool(name="outp", bufs=3))
        ps_proj = top.enter_context(tc.tile_pool(name="ps_proj", bufs=2, space="PSUM"))
        ps_scores = top.enter_context(
            tc.tile_pool(name="ps_scores", bufs=2, space="PSUM"))
        ps_av = top.enter_context(tc.tile_pool(name="ps_av", bufs=2, space="PSUM"))

        # ---- constants ----
        tanhg_sb = consts.tile([128, H], F32, tag="tanhg")
        if with_bias:
            bp_sb = consts.tile([1, DIM], BF16, tag="bp")
            nc.sync.dma_start(out=bp_sb, in_=bp_d)
            ones1 = consts.tile([1, 128], BF16, tag="ones1")
            nc.gpsimd.memset(ones1, 1.0)
        eps_t = consts.tile([128, 1], F32, tag="eps")
        nc.vector.memset(eps_t, LN_EPS)
        ident = consts.tile([128, 128], BF16, tag="ident")
        make_identity(nc, ident)

        # ---- persistent activations ----
        vw_sb = acts.tile([128, KT, H, HD + 1], BF16, tag="vw")  # [kv-part, kv-tile, h, d+1]
        attn_sb = acts.tile([128, NT, H, HD], BF16, tag="attn")  # [n-part, n-tile, h, d]

        # input loads, c-chunk granular; only xcatT is loaded up front --
        # wv/wp loads are emitted later, in consumption order
        xcatT_sb = ph1.tile([128, CC, KVP], BF16, tag="xcatT")
        # wv and wp share one slot: wv dies after the v projection, wp is
        # only needed from the output projection onwards
        wv_sb = ph1.tile([128, CC, DIM], BF16, tag="wvwp")
        # descriptor issue on an engine queue costs ~600ns each; spread the
        # startup-critical loads over the three DMA-capable queues, early
        # xcat chunks on the queues that clear their start barrier first
        w0q = wstream.tile([128, CC, 128], BF16, tag="w")
        nc.scalar.dma_start(out=w0q, in_=wq_re[:, :, 0:128])
        w0k = wstream.tile([128, CC, 128], BF16, tag="w")
        nc.scalar.dma_start(out=w0k, in_=wk_re[:, :, 0:128])
        xcat_q = [nc.gpsimd, nc.gpsimd, nc.gpsimd, nc.gpsimd,
                  nc.scalar, nc.sync, nc.sync, nc.sync]
        for cc in range(CC):
            xcat_q[cc].dma_start(out=xcatT_sb[:, cc, :], in_=xcat_re[:, cc, :])

        # ---- q/k projections interleaved with their dependent head pairs,
        # so ScalarE (exp) fills while PE still runs projections ----
        last_rows = KV - (KT - 1) * 128  # 77
        ksplits = [(0, 512), (512, 512), (1024, KV - 1024)]

        def emit_vproj(kvts):
            # v projection into vw (head-interleaved), natural [kv, o] layout
            for kvt in kvts:
                for half in range(2):
                    ps = ps_proj.tile([128, 512], F32, tag="ps")
                    for cc in range(CC):
                        nc.tensor.matmul(
                            ps,
                            xcatT_sb[:, cc, kvt * 128:(kvt + 1) * 128],
                            wv_sb[:, cc, half * 512:(half + 1) * 512],
                            start=(cc == 0),
                            stop=(cc == CC - 1),
                        )
                    nc.vector.tensor_copy(
                        vw_sb[:, kvt, half * 8:(half + 1) * 8, 0:HD],
                        ps.rearrange("p (h d) -> p h d", d=HD),
                    )

        def emit_qk(ot, wtq=None, wtk=None):
            qt = qkp.tile([128, N], BF16, tag="qt")
            kt = qkp.tile([128, KVP], BF16, tag="kt")
            # pad keys (kv 1101:1152) are zero; scores psum partitions for
            # them are never read by the exp, but zero them for the checker
            nc.gpsimd.memset(kt[:, KV:KVP], 0.0)
            if wtq is None:
                wtq = wstream.tile([128, CC, 128], BF16, tag="w")
                nc.sync.dma_start(out=wtq, in_=wq_re[:, :, ot * 128:(ot + 1) * 128])
            for half in range(2):
                ps = ps_proj.tile([128, 512], F32, tag="ps")
                for cc in range(CC):
                    nc.tensor.matmul(
                        ps,
                        wtq[:, cc, :],
                        xcatT_sb[:, cc, P + half * 512: P + (half + 1) * 512],
                        start=(cc == 0),
                        stop=(cc == CC - 1),
                    )
                nc.vector.tensor_copy(qt[:, half * 512:(half + 1) * 512], ps)
            if wtk is None:
                wtk = wstream.tile([128, CC, 128], BF16, tag="w")
                nc.sync.dma_start(out=wtk, in_=wk_re[:, :, ot * 128:(ot + 1) * 128])
            for off, width in ksplits:
                ps = ps_proj.tile([128, 512], F32, tag="ps")
                for cc in range(CC):
                    nc.tensor.matmul(
                        ps[:, :width],
                        wtk[:, cc, :],
                        xcatT_sb[:, cc, off:off + width],
                        start=(cc == 0),
                        stop=(cc == CC - 1),
                    )
                nc.vector.tensor_copy(kt[:, off:off + width], ps[:, :width])
            return qt, kt

        def emit_scores_pair(qt, kt):
            # Scores for the even/odd head pair.  Both heads of a (kv-tile,
            # n-half) share ONE [128,1024] psum tile: even head -> cols
            # 0:512 (bank A) on PE row-tile 0, odd head -> cols 512:1024
            # (bank B) on row-tile 64.  Disjoint row groups + disjoint psum
            # banks + a shared psum-reuse dependency means the two matmuls
            # issue back-to-back and execute concurrently on the PE.
            # E layout: [kv-part, kv-tile, n-half, head, 512].
            e2 = epool.tile([128, KT, 2, 2, 512], BF16, tag="e")
            nc.gpsimd.memset(e2[:, KT - 1], 0.0)
            # half-major order so the AV of n-tiles 0-3 (which only needs
            # half 0) can start as soon as half 0's exps have drained
            for half in range(2):
                for kvt in range(KT):
                    rows = last_rows if kvt == KT - 1 else 128
                    ps = ps_scores.tile([128, 1024], F32, tag="pss")
                    nc.tensor.matmul(
                        ps[:, 0:512],
                        kt[0:64, kvt * 128:(kvt + 1) * 128],
                        qt[0:64, half * 512:(half + 1) * 512],
                        start=True, stop=True,
                    )
                    nc.tensor.matmul(
                        ps[:, 512:1024],
                        kt[64:128, kvt * 128:(kvt + 1) * 128],
                        qt[64:128, half * 512:(half + 1) * 512],
                        start=True, stop=True,
                    )
                    nc.scalar.activation(
                        e2[:rows, kvt, half], ps[:rows], AF.Exp,
                        bias=0.0, scale=0.125)
            # first key column is gated separately; one memset per n-half so
            # the AV of n-tiles 0-3 only depends on half 0's exps
            nc.gpsimd.memset(e2[0:1, 0, 0], 0.0)
            nc.gpsimd.memset(e2[0:1, 0, 1], 0.0)
            return e2

        def eslice(e2, hh, kvt, nt):
            q, r = divmod(nt, 4)
            return e2[:, kvt, q, hh, r * 128:(r + 1) * 128]

        def emit_gv0(h):
            gv0 = tpool.tile([128, HD], BF16, tag="gv0")
            nc.gpsimd.partition_broadcast(gv0, vw_sb[0:1, 0, h, 0:HD])
            gv0s = tpool.tile([128, HD], F32, tag="gv0s")
            nc.vector.tensor_scalar_mul(gv0s, gv0, tanhg_sb[:, h:h + 1])
            return gv0s

        def emit_av_nt(h, e2, hh, nt, gv0s, alt, fixup_engine=None):
            if alt and nt % 2 == 1:
                avp = ps_proj.tile([128, HD + 1], F32, tag="ps")
            else:
                avp = ps_av.tile([128, HD + 1], F32, tag="avp")
            for kvt in range(KT):
                nc.tensor.matmul(
                    avp,
                    eslice(e2, hh, kvt, nt),
                    vw_sb[:, kvt, h, :],
                    start=(kvt == 0),
                    stop=(kvt == KT - 1),
                )
            rs = tpool.tile([128, 1], F32, tag="rs")
            nc.vector.reciprocal(rs, avp[:, HD:HD + 1])
            (fixup_engine or nc.vector).scalar_tensor_tensor(
                out=attn_sb[:, nt, h, :],
                in0=avp[:, 0:HD],
                scalar=rs,
                in1=gv0s,
                op0=OP.mult,
                op1=OP.add,
            )

        def emit_head_tail(h, e2, hh, alt=False):
            """Everything after E for head h: gate prep, AV + fixup.
            alt=True additionally cycles the (by-then idle) proj psum pool
            for deeper AV pipelining on the final head pairs."""
            gv0s = emit_gv0(h)
            for nt in range(NT):
                emit_av_nt(h, e2, hh, nt, gv0s, alt)

        # ---- LayerNorm per n-tile (consumed later by the projection) ----
        def emit_ln(nt):
            xa = attn_sb[:, nt].rearrange("p h d -> p (h d)")
            xs = xa.rearrange("p (s f) -> p s f", f=512)
            stats = tpool.tile([128, 2, 6], F32, tag="stats")
            for s in range(2):
                nc.vector.bn_stats(stats[:, s, :], xs[:, s, :])
            mv = tpool.tile([128, 2], F32, tag="mv")
            nc.vector.bn_aggr(mv, stats)
            rstd = tpool.tile([128, 1], F32, tag="rstd")
            nc.scalar.activation(rstd, mv[:, 1:2], AF.Sqrt, bias=eps_t, scale=1.0)
            nc.vector.reciprocal(rstd, rstd)
            L_t = qkp.tile([128, DIM], BF16, tag="qt")
            if with_bias:
                # ln_g/ln_b are folded into Wp/bp: L = (x - mu) * rstd
                nc.vector.tensor_scalar(
                    out=L_t, in0=xa, scalar1=mv[:, 0:1], scalar2=rstd,
                    op0=OP.subtract, op1=OP.mult,
                )
                return L_t, None
            # L = x - mu only; rstd is applied as the psum-evacuation scale
            # (scalar per n row distributes over the contraction), so the
            # transposes/matmuls don't wait on the sqrt -- whose activation
            # table load can only happen after the last exp
            nc.vector.tensor_scalar_sub(L_t, xa, mv[:, 0:1])
            return L_t, rstd

        # pair 0's scores/exp are hoisted before the v projection so ScalarE
        # starts as early as possible
        # software pipeline: scores/exp run one head-pair ahead of the
        # AV/fixup tails so ScalarE never starves
        pend = []
        qt0, kt0 = emit_qk(0, w0q, w0k)
        # split sync/scalar: one queue alone delivers the last chunk too
        # late for the v-projection (keep gpsimd free for the e2 memsets;
        # the scalar queue's exp stream only starts at the first pair)
        for cc in range(CC):
            dmae = nc.sync if cc % 2 == 0 else nc.scalar
            dmae.dma_start(out=wv_sb[:, cc, :], in_=wv_re[:, cc, :])
        pend.append(emit_scores_pair(qt0, kt0))
        qt, kt = emit_qk(1)
        pend.append(emit_scores_pair(qt, kt))
        # ones column for the row-sum S (E rows for kv=0/pad are zeroed);
        # disjoint from the v-projection's columns, so set it up front
        nc.gpsimd.memset(vw_sb[:, :, :, HD:HD + 1], 1.0)
        # tanh(gate) broadcast: first consumer is the head tails ~90us in,
        # so keep it off the startup-critical queues (and off scalar, whose
        # queue is the exp stream by this point)
        nc.sync.dma_start(out=tanhg_sb, in_=tanhg_d.to_broadcast([128, H]))
        emit_vproj(range(KT))
        wp_sb = ph1.tile([128, CC, DIM], BF16, tag="wvwp")
        for cc in range(CC):
            nc.sync.dma_start(out=wp_sb[:, cc, :], in_=wp_re[:, cc, :])
        done = 0
        for ot in range(2, OT):
            qt, kt = emit_qk(ot)
            pend.append(emit_scores_pair(qt, kt))
            ep = pend.pop(0)
            emit_head_tail(2 * done, ep, 0, alt=(ot >= OT - 2))
            emit_head_tail(2 * done + 1, ep, 1, alt=(ot >= OT - 2))
            done += 1
        def emit_outproj(nt, L_t, rstd):
            # transpose LN rows then project: out[n, o] = L @ Wp'.T + bp'
            pp0 = ps_proj.tile([128, 512], F32, tag="ps")
            pp1 = ps_proj.tile([128, 512], F32, tag="ps")
            for cc in range(CC):
                # ps_av only: ps_scores' slots still have WAR dependencies
                # on the final pair's exp, which would stall early n-tiles
                pst = ps_av.tile([128, 128], BF16, tag="avp")
                nc.tensor.transpose(
                    pst, L_t[:, cc * 128:(cc + 1) * 128], ident
                )
                ltc = ltp.tile([128, 128], BF16, tag="ltc")
                # DVE only: ScalarE's in-order queue is saturated with exps
                # while the first projections run -- a copy waiting on its
                # transpose there would stall the whole exp stream
                nc.vector.tensor_copy(ltc, pst)
                nc.tensor.matmul(
                    pp0, ltc, wp_sb[:, cc, 0:512],
                    start=(cc == 0), stop=(not with_bias and cc == CC - 1),
                )
                nc.tensor.matmul(
                    pp1, ltc, wp_sb[:, cc, 512:1024],
                    start=(cc == 0), stop=(not with_bias and cc == CC - 1),
                )
            if with_bias:
                # bias as rank-1 accumulation (PSUM is not a legal DMA
                # source, so stage through SBUF)
                nc.tensor.matmul(pp0, ones1, bp_sb[:, 0:512], start=False, stop=True)
                nc.tensor.matmul(pp1, ones1, bp_sb[:, 512:1024], start=False, stop=True)
            ot0 = opool.tile([128, 512], F32, tag="ot")
            ot1 = opool.tile([128, 512], F32, tag="ot")
            if rstd is None:
                nc.scalar.copy(out=ot0, in_=pp0)
                nc.scalar.copy(out=ot1, in_=pp1)
            else:
                nc.scalar.activation(ot0, pp0, AF.Copy, bias=0.0, scale=rstd)
                nc.scalar.activation(ot1, pp1, AF.Copy, bias=0.0, scale=rstd)
            nc.sync.dma_start(out=out_d[nt * 128:(nt + 1) * 128, 0:512], in_=ot0)
            nc.sync.dma_start(out=out_d[nt * 128:(nt + 1) * 128, 512:1024], in_=ot1)

        ep = pend.pop(0)
        emit_head_tail(2 * done, ep, 0, alt=True)
        emit_head_tail(2 * done + 1, ep, 1, alt=True)
        done += 1
        # final pair: nt-major AV with the LayerNorm chain and the output
        # projection of the previous n-tile interleaved, so the PE stays on
        # projection matmuls while the DVE runs the LN chain of n-tile nt.
        # AV psum comes only from ps_av here; ps_proj is cycling pp0/pp1.
        ep = pend.pop(0)
        gv0s_e = emit_gv0(2 * done)
        gv0s_o = emit_gv0(2 * done + 1)
        # depth-2 pipeline: the LN chain of n-tile nt has the AV of nt+1
        # plus the projection of nt-1 of PE time to complete before the
        # projection of nt needs it -- depth 1 stalls the PE on the DVE
        # every n-tile and drops it out of max p-state
        L_q = []
        for nt in range(NT):
            emit_av_nt(2 * done, ep, 0, nt, gv0s_e, alt=False)
            emit_av_nt(2 * done + 1, ep, 1, nt, gv0s_o, alt=False)
            L_t, rstd = emit_ln(nt)
            L_q.append((nt, L_t, rstd))
            if len(L_q) > 2:
                emit_outproj(*L_q.pop(0))
        for item in L_q:
            emit_outproj(*item)


def build_program(with_bias=False):
    key = ("nc", with_bias)
    if key in _CACHE:
        return _CACHE[key]
    nc = bacc.Bacc("TRN2", target_bir_lowering=False, debug=False, num_devices=8,
                   enable_partition_id=False)
    with tile.TileContext(nc) as tc:
        _emit(tc, with_bias)
    nc.compile()
    _CACHE[key] = nc
    return nc


def prep_inputs(x, x_text, Wq, Wk, Wv, gate, ln_g, ln_b, Wp, bp):
    """Host-side sharding/layout prep. Returns the 8 per-core input maps."""
    bf = ml_dtypes.bfloat16
    x = np.asarray(x, np.float32)
    x_text = np.asarray(x_text, np.float32)
    xcat = np.concatenate([x_text, x], axis=1)          # [B, KV, DIM]
    xcatT = np.zeros((B, DIM, KVP), np.float32)
    xcatT[:, :, :KV] = xcat.transpose(0, 2, 1)
    xcatT = xcatT.astype(bf)
    wqT = np.ascontiguousarray(np.asarray(Wq, np.float32).T).astype(bf)
    wkT = np.ascontiguousarray(np.asarray(Wk, np.float32).T).astype(bf)
    wvT = np.ascontiguousarray(np.asarray(Wv, np.float32).T).astype(bf)
    # fold LayerNorm affine into the output projection:
    #   (L*g + b) @ Wp.T + bp == L @ (Wp*g).T + (bp + Wp @ b)
    Wp = np.asarray(Wp, np.float32)
    g = np.asarray(ln_g, np.float32).reshape(DIM)
    bvec = np.asarray(ln_b, np.float32).reshape(DIM)
    Wpf = Wp * g[None, :]
    bpf = np.asarray(bp, np.float32).reshape(DIM) + Wp @ bvec
    wpT = np.ascontiguousarray(Wpf.T).astype(bf)
    tanhg = np.tanh(np.asarray(gate, np.float32)).reshape(1, H).astype(np.float32)
    bp_bf = bpf.reshape(1, DIM).astype(bf)
    in_maps = []
    for b in range(B):
        in_maps.append({
            "xcatT": np.ascontiguousarray(xcatT[b]),
            "wqT": wqT, "wkT": wkT, "wvT": wvT, "wpT": wpT,
            "tanhg": tanhg, "bp_bf": bp_bf,
        })
    return in_maps


def kernel(**inputs):
    global LAST_EXEC_NS
    in_maps = prep_inputs(**inputs)
    with_bias = bool(np.any(in_maps[0]["bp_bf"]))
    nc = build_program(with_bias)
    trace = bool(int(os.environ.get("BASS_TRACE_RUN", "0")))
    res = run_bass_kernel_spmd(
        nc, in_maps, core_ids=list(range(8)), trace=trace,
    )
    LAST_EXEC_NS = res.exec_time_ns
    out = np.stack([r["out"] for r in res.results], axis=0)
    return out.astype(np.float32)
